# revision 20
# baseline (speedup 1.0000x reference)
"""Trainium2 Bass kernel for nn_Attention_7602092114471.

Full multi-head attention block:
  qkv = x @ w_qkv.T ; split q,k,v into 12 heads of d=64
  q = rope(q * d**-0.5) ; k = rope(k)   (lucidrains interleaved RoPE)
  attn = softmax(q @ k.T) ; out = (attn @ v) reassembled, @ w_proj.T + b_proj

Shapes: x [2, 2048, 768], w_qkv [2304, 768], w_proj [768, 768], b_proj [768].

Sharding: 24 (batch, head) pairs -> 8 cores x 3 heads. Core c handles batch
c//4, heads {3g, 3g+1, 3g+2} with g = c%4. Each core computes its heads'
q/k/v projections, attention, and a partial output projection over its
3 heads' feature columns. The host sums the 4 partial projections per batch
(the tensor-parallel all-reduce, done on host during unshard) and adds bias.

v4 design (v1 218us -> v2 209 -> v3 202 -> this). All matmuls bf16.
  * DMA: inputs are pre-packed on the host so every transfer has >=1.5KB
    contiguous runs per partition row (v3's 256-512B runs collapsed
    per-ring bandwidth ~4x and starved both the prologue and the strip-0
    filler chains). x is packed [128, strip, kt, 512]; w is packed
    block-major [128, block, kt, cols]; transfers are split across rings
    (partition quarters / kt pairs) so the prologue set lands in ~4us.
  * One ACT table set for the whole kernel: the exp-only and ln-only
    table entries are masked during compilation so every Exp AND Ln ACT
    resolves to natural_log_exp_and_others -> exactly one ACT_TABLE_LOAD
    (v2 thrashed 13 loads); softmax 1/L runs on DVE reciprocal in-loop
    and on ScalarE Ln/Exp(-x) in the tail where ScalarE is idle.
  * No swap-projection chains: rope's rotate-half partner via 4 cross-
    partition [32,512] bf16 SBUF copies (DVE 4x rate) off a bf16 copy of
    the projection PSUM; sin sign pattern baked into the host table.
  * v produced token-major directly (x-block stationary, v weight columns
    moving) -> [tok, 3*64] PSUM -> one scatter per block. No PE
    transposes; ones columns prefilled by one gpsimd memset.
  * Scores: one [128, 3, 512] PSUM group (3 banks, double-buffered) per
    key block = 3 MMs + ONE 1536-col exp ACT for all 3 heads.
  * Strip 3's PV runs inside the score stream: h0 steals a retiring
    score-group PSUM buffer after jb14, h1/h2 rotate through the work
    banks as late fillers; only the three jb15 matmuls, the tail norms
    and the last projection remain after the final exp.
"""

import os

import numpy as np

import concourse.bass as bass
import concourse.mybir as mybir
import concourse.tile as tile
from concourse import bacc, bass_utils

# Problem constants (hardcoded per contract; kernel.py must be self-contained).
B = 2
N = 2048
C = 768
H = 12
D = 64
ROPE_THETA = 10000.0
NCORES = 8

F32 = mybir.dt.float32
BF16 = mybir.dt.bfloat16

IS = 512                  # strip width (projections and attention i-strips)
NSTRIP = N // IS          # 4
NJB = N // 128            # 16 key blocks
NJP = NJB // 2            # 8 key-block pairs (v_sb/e pairing layout)
KT = C // 128             # 6 contraction tiles for the projections
EXP_BIAS = -2.0           # constant shift inside exp; cancels in normalization

# packed-w block offsets (in the [128, KT*cols] host layout)
WK_OFF = 0                # k0|k1   KT*128
WQK2_OFF = 768            # q2|k2   KT*128
WQ_OFF = 1536             # q0|q1   KT*128
WV_OFF = 2304             # v0|v1|v2  KT*192
WP_COLS = 3456

DEBUG_DUMP = os.environ.get("K_DEBUG_DUMP", "0") == "1"

# ACT table sets whose presence would split Exp and Ln across different
# tables (one reload per switch); masking them makes both resolve to
# natural_log_exp_and_others.
_MASK_ACT_SETS = ("exp_and_others", "natural_log", "exp_and_friends")


def build_nc():
    """Build the per-core Bass module (same NEFF runs SPMD on all 8 cores)."""
    import concourse.bacc as bacc_mod

    orig_tables = bacc_mod.get_activation_tables

    def patched_tables(arch):
        t = orig_tables(arch)
        return {
            name: (set() if name in _MASK_ACT_SETS else funcs)
            for name, funcs in t.items()
        }

    nc = bacc.Bacc(
        "TRN2",
        target_bir_lowering=False,
        debug=False,
        enable_asserts=False,
    )

    xP = nc.dram_tensor("xP", [128, NSTRIP * KT * IS], BF16, kind="ExternalInput").ap()
    wP = nc.dram_tensor("wP", [128, WP_COLS], BF16, kind="ExternalInput").ap()
    wp = nc.dram_tensor("wp", [256, C], BF16, kind="ExternalInput").ap()
    cosT = nc.dram_tensor("cosT", [128, N], BF16, kind="ExternalInput").ap()
    sinT = nc.dram_tensor("sinT", [128, N], BF16, kind="ExternalInput").ap()
    outT = nc.dram_tensor("outT", [C, N], BF16, kind="ExternalOutput").ap()
    dbg = None
    if DEBUG_DUMP:
        dbg = {
            nm: nc.dram_tensor(f"dbg_{nm}", shp, dt, kind="ExternalOutput").ap()
            for nm, shp, dt in [
                ("q01", [128, N], BF16), ("k01", [128, N], BF16),
                ("qk2d", [128, N], BF16),
                ("v_sb", [128, NJP * 2 * 384], BF16),
                ("e0", [128, NJP * 2 * 3 * IS], BF16),
                ("e1", [128, NJP * 2 * 3 * IS], BF16),
                ("P0", [128, N], BF16), ("P1", [128, N], BF16),
            ]
        }

    bacc_mod.get_activation_tables = patched_tables
    try:
        with tile.TileContext(nc) as tc:
            _kernel_body(tc, nc, xP, wP, wp, cosT, sinT, outT, dbg)
        nc.compile()
    finally:
        bacc_mod.get_activation_tables = orig_tables
    return nc


def _kernel_body(tc, nc, xP, wP, wp, cosT, sinT, outT, dbg=None):
    import contextlib

    Exp = mybir.ActivationFunctionType.Exp
    Ln = mybir.ActivationFunctionType.Ln

    ctx = contextlib.ExitStack()
    with ctx:
        persist = ctx.enter_context(tc.tile_pool(name="persist", bufs=1))
        rope_pool = ctx.enter_context(tc.tile_pool(name="rope", bufs=2))
        nrm = ctx.enter_context(tc.tile_pool(name="nrm", bufs=1))
        prout = ctx.enter_context(tc.tile_pool(name="prout", bufs=6))
        attnA = ctx.enter_context(tc.tile_pool(name="attnA", bufs=1))
        # PSUM: 3-bank score groups double-buffered (6) + 2 work banks
        stsp = ctx.enter_context(tc.tile_pool(name="sts", bufs=2, space="PSUM"))
        wkps = ctx.enter_context(tc.tile_pool(name="wkps", bufs=2, space="PSUM"))

        # ---- persistent SBUF tensors -------------------------------------
        q01 = persist.tile([128, N], BF16, name="q01")
        k01 = persist.tile([128, N], BF16, name="k01")
        qk2d = persist.tile([128, N], BF16, name="qk2d")  # q2 rows 0:64 | k2 64:128
        k2lo = persist.tile([64, N], BF16, name="k2lo")   # k2 at base partition 0
        # (v | ones) stationary groups, key-block-paired layout
        v_sb = persist.tile([128, NJP, 2, 3, 128], BF16, name="v_sb")
        P0 = persist.tile([128, N], BF16, name="P0")  # heads h0 | h1
        P1 = persist.tile([128, N], BF16, name="P1")  # h2 duplicated
        wp_sb = persist.tile([128, 2, C], BF16, name="wp_sb")
        bias_sb = persist.tile([128, 1], F32, name="bias_sb")
        warm = persist.tile([128, 64], F32, name="warm")
        warm_o = persist.tile([128, 64], F32, name="warm_o")

        e_all = [
            attnA.tile([128, NJP, 2, 3, IS], BF16, name="e0"),
            attnA.tile([128, NJP, 2, 3, IS], BF16, name="e1"),
        ]

        ph1_stack = contextlib.ExitStack()
        ph1 = ph1_stack.enter_context(tc.tile_pool(name="ph1", bufs=1))
        wk_sb = ph1.tile([128, KT, 128], BF16, name="wk_sb")
        wqk2_sb = ph1.tile([128, KT, 128], BF16, name="wqk2_sb")
        wq_sb = ph1.tile([128, KT, 128], BF16, name="wq_sb")
        wv_sb = ph1.tile([128, KT, 192], BF16, name="wv_sb")
        cos_sb = ph1.tile([128, N], BF16, name="cos_sb")
        sin_sb = ph1.tile([128, N], BF16, name="sin_sb")
        x_sb = ph1.tile([128, NSTRIP, KT, IS], BF16, name="x_sb")

        xPr = xP.rearrange("p (s k c) -> p s k c", s=NSTRIP, k=KT)

        def dma_wblock(dst, off, wd, kt_split=False):
            src = wP[:, off:off + KT * wd].rearrange("p (k c) -> p k c", k=KT)
            if kt_split:
                for j in range(KT // 2):
                    js = slice(2 * j, 2 * j + 2)
                    nc.sync.dma_start(dst[:, js], src[:, js])
                return
            for q in range(4):
                qs = slice(q * 32, (q + 1) * 32)
                nc.sync.dma_start(dst[qs], src[qs])

        def dma_xstrip(s, split_part=False):
            for j in range(KT // 2):
                js = slice(2 * j, 2 * j + 2)
                if split_part:
                    for hx in range(4):
                        hs = slice(hx * 32, (hx + 1) * 32)
                        nc.sync.dma_start(x_sb[hs, s, js], xPr[hs, s, js])
                else:
                    nc.sync.dma_start(x_sb[:, s, js], xPr[:, s, js])

        def dma_trig(s):
            ss = slice(s * IS, (s + 1) * IS)
            for hx in range(2):
                hs = slice(hx * 64, (hx + 1) * 64)
                nc.sync.dma_start(cos_sb[hs, ss], cosT[hs, ss])
                nc.sync.dma_start(sin_sb[hs, ss], sinT[hs, ss])

        # prologue-critical first, in chain consumption order: the k01
        # chain's kt-pair weights + x quarters land on the earliest rings so
        # its matmuls pace with DMA arrival instead of waiting for the set
        wkr = wP[:, WK_OFF:WK_OFF + KT * 128].rearrange("p (k c) -> p k c", k=KT)
        for j in range(KT // 2):
            js = slice(2 * j, 2 * j + 2)
            nc.sync.dma_start(wk_sb[:, js], wkr[:, js])
            for hx in range(4):
                hs = slice(hx * 32, (hx + 1) * 32)
                nc.sync.dma_start(x_sb[hs, 0, js], xPr[hs, 0, js])
        dma_wblock(wqk2_sb, WQK2_OFF, 128, kt_split=True)
        dma_wblock(wq_sb, WQ_OFF, 128, kt_split=True)
        dma_trig(0)
        for s in range(1, NSTRIP):
            dma_xstrip(s)
            dma_trig(s)
        dma_wblock(wv_sb, WV_OFF, 192)
        wpr = wp.rearrange("(o p) f -> p o f", p=128)
        for hx in range(2):
            hs = slice(hx * 64, (hx + 1) * 64)
            nc.sync.dma_start(wp_sb[hs], wpr[hs])

        nc.vector.memset(bias_sb, EXP_BIAS)
        nc.vector.memset(warm, 0.0)
        # early ACT table load during the DMA window
        nc.scalar.activation(out=warm_o, in_=warm, func=Exp)
        # non-blocking PE warmup: ~3us of zero matmuls during the DMA wait
        # keeps the HAM clock gate from starting the first chains at K=4/8
        wmt = wkps.tile([128, IS], F32, name="wk", tag="wk")
        for _ in range(21):
            nc.tensor.matmul(wmt[:64, 0:64], warm[:, 0:64], warm[:, 0:64],
                             start=True, stop=True)
        # ones columns of the (v | ones) PV groups
        nc.gpsimd.memset(v_sb[:, :, :, :, 64:128], 1.0)

        # ---- projection chain + rope ------------------------------------
        def rope_group(dst, wsrc, s, pre_scalar=False, pt=None, mul_eng=None):
            """One 128-feature projection chain + rope into dst[:, strip s].

            rotate-half partner via 4 cross-partition bf16 copies; sin sign
            pattern ([-sin;+sin] per 32-row half) baked into sinT."""
            ss = slice(s * IS, (s + 1) * IS)
            if pt is None:
                pt = wkps.tile([128, IS], F32, name="wk", tag="wk")
                for kt in range(KT):
                    nc.tensor.matmul(
                        pt, wsrc[:, kt, :], x_sb[:, s, kt, :],
                        start=(kt == 0), stop=(kt == KT - 1),
                    )
            qpre = rope_pool.tile([128, IS], BF16, name="qpre", tag="qpre")
            if pre_scalar:
                nc.scalar.copy(out=qpre, in_=pt)
            else:
                nc.vector.tensor_copy(out=qpre, in_=pt)
            qps = rope_pool.tile([128, IS], BF16, name="qps", tag="qps")
            for (a, b) in ((0, 32), (32, 0), (64, 96), (96, 64)):
                nc.vector.tensor_copy(out=qps[a:a + 32, :], in_=qpre[b:b + 32, :])
            tmp1 = rope_pool.tile([128, IS], BF16, name="tmp1", tag="tmp1")
            tmp2 = rope_pool.tile([128, IS], BF16, name="tmp2", tag="tmp2")
            eng = mul_eng or nc.vector
            eng.tensor_mul(out=tmp1, in0=qpre, in1=cos_sb[:, ss])
            eng.tensor_mul(out=tmp2, in0=qps, in1=sin_sb[:, ss])
            eng.tensor_add(out=dst[:, ss], in0=tmp1, in1=tmp2)
            if dst is qk2d:
                # matmul needs lhsT/rhs on the same base partition: keep a
                # base-0 copy of k2 for the h2 score matmuls
                nc.vector.tensor_copy(out=k2lo[:, ss], in_=qk2d[64:128, ss])

        def v_block(tb):
            """v for one 128-token block, token-major: x-block stationary,
            v weight columns moving -> [tok, 3*64] -> scatter into the
            key-block-paired v_sb layout."""
            s, sb = divmod(tb, 4)
            pt = wkps.tile([128, IS], F32, name="wk", tag="wk")
            for kt in range(KT):
                nc.tensor.matmul(
                    pt[:, 0:192],
                    x_sb[:, s, kt, sb * 128:(sb + 1) * 128],
                    wv_sb[:, kt, :],
                    start=(kt == 0), stop=(kt == KT - 1),
                )
            nc.vector.tensor_copy(
                out=v_sb[:, tb // 2, tb % 2, :, 0:64],
                in_=pt[:, 0:192].rearrange("p (h x) -> p h x", h=3),
            )

        # ---- scores + exp -----------------------------------------------
        def score_group(s, jb):
            ss = slice(s * IS, (s + 1) * IS)
            jbs = slice(jb * 128, (jb + 1) * 128)
            st = stsp.tile([128, 3, IS], F32, name="st", tag="st")
            nc.tensor.matmul(st[:, 0, :], k01[0:64, jbs], q01[0:64, ss],
                             start=True, stop=True)
            nc.tensor.matmul(st[:, 1, :], k01[64:128, jbs], q01[64:128, ss],
                             start=True, stop=True)
            nc.tensor.matmul(st[:, 2, :], k2lo[:, jbs], qk2d[0:64, ss],
                             start=True, stop=True)
            if s == 3 and jb == NJB - 1:
                for h in range(3):
                    nc.scalar.activation(
                        out=e_all[s % 2][:, jb // 2, jb % 2, h], in_=st[:, h, :],
                        func=Exp, bias=bias_sb[:, :],
                    )
            else:
                nc.scalar.activation(
                    out=e_all[s % 2][:, jb // 2, jb % 2], in_=st,
                    func=Exp, bias=bias_sb[:, :],
                )

        # ---- PV + normalization -----------------------------------------
        pvst = {}
        ALLGM = [(g, m) for g in range(NJP) for m in range(2)]

        def pv_mms(ps, h, pv, gms):
            et = e_all[ps % 2]
            for (g, m) in gms:
                nc.tensor.matmul(
                    pv, v_sb[:, g, m, h, :], et[:, g, m, h, :],
                    start=(g == 0 and m == 0),
                    stop=(g == NJP - 1 and m == 1),
                )

        def pv_start(ps, h):
            pv = wkps.tile([128, IS], F32, name="wk", tag="wk")
            pvst[(ps, h)] = pv
            pv_mms(ps, h, pv, ALLGM[:8])

        def pv_end(ps, h):
            pv_mms(ps, h, pvst[(ps, h)], ALLGM[8:])

        def norm01(ps, tail=False):
            """h0/h1: numerators+denominators copied out packed (fast PSUM
            bank release), 1/L via ScalarE Ln -> Exp(-x) (the Ln/Exp pair
            lives in the same ACT table set as the score exps)."""
            ss = slice(ps * IS, (ps + 1) * IS)
            pv0 = pvst.pop((ps, 0))
            pv1 = pvst.pop((ps, 1))
            r01 = nrm.tile([128, IS], F32, name="r01", tag="r01")
            lt = nrm.tile([128, IS], F32, name="lt", tag="lt")
            if tail:
                # no bank pressure after the last strip: read PSUM directly
                cn, cd = None, None
                nc.scalar.activation(out=lt[0:64, :], in_=pv0[64:128, :], func=Ln)
                nc.scalar.activation(out=lt[64:128, :], in_=pv1[64:128, :], func=Ln)
            else:
                cn = nrm.tile([128, IS], F32, name="cn", tag="cn")
                cd = nrm.tile([128, IS], F32, name="cd", tag="cd")
                nc.vector.tensor_copy(out=cn[0:64, :], in_=pv0[0:64, :])
                nc.vector.tensor_copy(out=cd[0:64, :], in_=pv0[64:128, :])
                nc.vector.tensor_copy(out=cn[64:128, :], in_=pv1[0:64, :])
                nc.vector.tensor_copy(out=cd[64:128, :], in_=pv1[64:128, :])
                nc.scalar.activation(out=lt, in_=cd, func=Ln)
            nc.scalar.activation(out=r01, in_=lt, func=Exp, scale=-1.0)
            n0 = pv0[0:64, :] if tail else cn[0:64, :]
            n1 = pv1[0:64, :] if tail else cn[64:128, :]
            nc.vector.tensor_mul(out=P0[0:64, ss], in0=n0, in1=r01[0:64, :])
            nc.vector.tensor_mul(out=P0[64:128, ss], in0=n1, in1=r01[64:128, :])

        def norm2(ps, tail=False):
            ss = slice(ps * IS, (ps + 1) * IS)
            pv2 = pvst.pop((ps, 2))
            r2 = nrm.tile([64, IS], F32, name="r2", tag="r2")
            t2 = nrm.tile([64, IS], F32, name="t2", tag="t2")
            if tail:
                cn2 = None
                nc.scalar.activation(out=t2, in_=pv2[64:128, :], func=Ln)
            else:
                cn2 = nrm.tile([64, IS], F32, name="cn2", tag="cn2")
                cd2 = nrm.tile([64, IS], F32, name="cd2", tag="cd2")
                nc.vector.tensor_copy(out=cn2, in_=pv2[0:64, :])
                nc.vector.tensor_copy(out=cd2, in_=pv2[64:128, :])
                nc.scalar.activation(out=t2, in_=cd2, func=Ln)
            nc.scalar.activation(out=r2, in_=t2, func=Exp, scale=-1.0)
            n2 = pv2[0:64, :] if tail else cn2
            nc.vector.tensor_mul(out=P1[0:64, ss], in0=n2, in1=r2)
            nc.vector.tensor_copy(out=P1[64:128, ss], in_=P1[0:64, ss])

        def proj_obs(t, obs, alt_cast=False):
            ss = slice(t * IS, (t + 1) * IS)
            for ob in obs:
                obsl = slice(ob * 128, (ob + 1) * 128)
                pp = wkps.tile([128, IS], F32, name="wk", tag="wk")
                nc.tensor.matmul(pp, wp_sb[:, 0, obsl], P0[:, ss],
                                 start=True, stop=False)
                nc.tensor.matmul(pp, wp_sb[:, 1, obsl], P1[:, ss],
                                 start=False, stop=True)
                ot = prout.tile([128, IS], BF16, name="ot", tag="ot")
                if alt_cast and ob % 2 == 1:
                    nc.scalar.copy(out=ot, in_=pp)
                else:
                    nc.vector.tensor_copy(out=ot, in_=pp)
                for hx in range(2):
                    hs = slice(ob * 128 + hx * 64, ob * 128 + (hx + 1) * 64)
                    nc.sync.dma_start(outT[hs, ss], ot[hx * 64:(hx + 1) * 64, :])

        # ---- prologue: strip 0's own projections ------------------------
        # all three chains accumulate in one 3-bank score-group tile with
        # their matmuls interleaved by kt, so each fires as its x kt-pair
        # lands instead of serializing chain-after-chain on the in-order PE
        pst = stsp.tile([128, 3, IS], F32, name="st", tag="st")
        for kt in range(KT):
            for slot, wsrc in ((0, wk_sb), (1, wqk2_sb), (2, wq_sb)):
                nc.tensor.matmul(
                    pst[:, slot, :], wsrc[:, kt, :], x_sb[:, 0, kt, :],
                    start=(kt == 0), stop=(kt == KT - 1),
                )
        rope_group(k01, None, 0, pre_scalar=True, pt=pst[:, 0, :])
        rope_group(qk2d, None, 0, pre_scalar=True, pt=pst[:, 1, :],
                   mul_eng=nc.gpsimd)
        rope_group(q01, None, 0, pre_scalar=True, pt=pst[:, 2, :])

        # ---- strip 0: scores/exp with phase 1 as filler ------------------
        # k01/qk2 of strip t must land before score group jb=4t.
        s0_fillers = [
            lambda: rope_group(k01, wk_sb, 1),
            lambda: rope_group(qk2d, wqk2_sb, 1),
            lambda: rope_group(k01, wk_sb, 2),
            lambda: rope_group(qk2d, wqk2_sb, 2),
            lambda: rope_group(k01, wk_sb, 3),
            lambda: rope_group(qk2d, wqk2_sb, 3),
            lambda: v_block(0),
            lambda: v_block(1),
            lambda: rope_group(q01, wq_sb, 1),
            lambda: v_block(2),
            lambda: v_block(3),
            lambda: rope_group(q01, wq_sb, 2),
            lambda: v_block(4),
            lambda: v_block(5),
            lambda: v_block(6),
            lambda: v_block(7),
        ]
        for jb in range(NJB):
            score_group(0, jb)
            s0_fillers[jb]()

        # ---- strips 1..3 + PV/norm/proj fillers + tail -------------------
        def pv3_main(h):
            pv = wkps.tile([128, IS], F32, name="wk", tag="wk")
            pvst[(3, h)] = pv
            pv_mms(3, h, pv, ALLGM[:15])

        strip_fillers = {
            1: [
                lambda: rope_group(q01, wq_sb, 3),
                lambda: v_block(8), lambda: v_block(9),
                lambda: v_block(10), lambda: v_block(11),
                lambda: pv_start(0, 0),
                lambda: v_block(12), lambda: v_block(13),
                lambda: v_block(14), lambda: v_block(15),
                lambda: pv_end(0, 0),
                lambda: pv_start(0, 1), lambda: pv_end(0, 1),
                lambda: norm01(0),
                lambda: (pv_start(0, 2), pv_end(0, 2)),
                lambda: norm2(0),
            ],
            2: [
                lambda: pv_start(1, 0), lambda: pv_end(1, 0),
                lambda: pv_start(1, 1), lambda: pv_end(1, 1),
                lambda: norm01(1),
                lambda: pv_start(1, 2), lambda: pv_end(1, 2),
                lambda: norm2(1),
                lambda: proj_obs(0, [0, 1]),
                lambda: proj_obs(0, [2, 3]),
                lambda: proj_obs(0, [4, 5]),
            ],
            3: [
                lambda: pv_start(2, 0), lambda: pv_end(2, 0),
                lambda: pv_start(2, 1), lambda: pv_end(2, 1),
                lambda: norm01(2),
                lambda: pv_start(2, 2), lambda: pv_end(2, 2),
                lambda: norm2(2),
                lambda: proj_obs(1, [0, 1]),
                lambda: proj_obs(1, [2, 3]),
                lambda: proj_obs(1, [4, 5]),
                lambda: proj_obs(2, [0, 1]),
                lambda: proj_obs(2, [2, 3]),
                lambda: proj_obs(2, [4, 5]),
                lambda: pv3_main(1),
                lambda: pv3_main(2),
            ],
        }
        for s in range(1, NSTRIP):
            fillers = strip_fillers[s]
            fi = 0
            for jb in range(NJB):
                score_group(s, jb)
                if s == 3 and jb == 14:
                    # steal the score-group PSUM buffer retiring after
                    # ACT(13) so h0's PV overlaps the last score groups
                    st3 = stsp.tile([128, 3, IS], F32, name="st", tag="st")
                    pvst[(3, 0)] = st3[:, 0, :]
                    pv_mms(3, 0, pvst[(3, 0)], ALLGM[:15])
                if fi < len(fillers):
                    fillers[fi]()
                    fi += 1
            while fi < len(fillers):
                fillers[fi]()
                fi += 1
            if s == 1:
                ph1_stack.close()

        # tail: only the jb15 PV matmuls, tail norms, last projection
        pv_mms(3, 0, pvst[(3, 0)], [ALLGM[15]])
        pv_mms(3, 1, pvst[(3, 1)], [ALLGM[15]])
        pv_mms(3, 2, pvst[(3, 2)], [ALLGM[15]])
        norm01(3, tail=True)
        norm2(3, tail=True)
        # the score-group PSUM buffers are retired: 2 steals = 6 banks, so
        # the six tail projections never wait on a bank rotation
        stp = [stsp.tile([128, 3, IS], F32, name="st", tag="st") for _ in range(2)]
        ss3 = slice(3 * IS, 4 * IS)
        for ob in range(6):
            obsl = slice(ob * 128, (ob + 1) * 128)
            pp = stp[ob // 3][:, ob % 3, :]
            nc.tensor.matmul(pp, wp_sb[:, 0, obsl], P0[:, ss3],
                             start=True, stop=False)
            nc.tensor.matmul(pp, wp_sb[:, 1, obsl], P1[:, ss3],
                             start=False, stop=True)
            ot = prout.tile([128, IS], BF16, name="ot", tag="ot")
            if ob % 2 == 1:
                nc.scalar.copy(out=ot, in_=pp)
            else:
                nc.vector.tensor_copy(out=ot, in_=pp)
            for hx in range(2):
                hs = slice(ob * 128 + hx * 64, ob * 128 + (hx + 1) * 64)
                nc.sync.dma_start(outT[hs, ss3], ot[hx * 64:(hx + 1) * 64, :])

        if dbg is not None:
            nc.sync.dma_start(dbg["q01"], q01)
            nc.sync.dma_start(dbg["k01"], k01)
            nc.sync.dma_start(dbg["qk2d"], qk2d)
            nc.sync.dma_start(dbg["v_sb"], v_sb.rearrange("p a b c d -> p (a b c d)"))
            nc.sync.dma_start(dbg["e0"], e_all[0].rearrange("p a b c d -> p (a b c d)"))
            nc.sync.dma_start(dbg["e1"], e_all[1].rearrange("p a b c d -> p (a b c d)"))
            nc.sync.dma_start(dbg["P0"], P0)
            nc.sync.dma_start(dbg["P1"], P1)


# ---------------------------------------------------------------------------
# Host-side sharding / unsharding
# ---------------------------------------------------------------------------

def _rope_tables():
    inv_freq = 1.0 / (ROPE_THETA ** (np.arange(0, D, 2, dtype=np.float64) / D))
    ang = np.arange(N, dtype=np.float64)[None, :] * inv_freq[:, None]  # [32, N]
    cos64 = np.concatenate([np.cos(ang), np.cos(ang)], axis=0)
    sin64 = np.concatenate([-np.sin(ang), np.sin(ang)], axis=0)
    cosT = np.concatenate([cos64, cos64], axis=0)
    sinT = np.concatenate([sin64, sin64], axis=0)
    return cosT, sinT  # [128, N] float64


def _bf(a):
    import ml_dtypes

    return np.ascontiguousarray(a).astype(ml_dtypes.bfloat16)


def make_core_inputs(x, w_qkv, w_proj):
    """Build the 8 per-core input dicts from full inputs."""
    x = np.asarray(x, dtype=np.float32)
    w_qkv = np.asarray(w_qkv, dtype=np.float32)
    w_proj = np.asarray(w_proj, dtype=np.float32)

    cosT, sinT = _rope_tables()
    cosT, sinT = _bf(cosT), _bf(sinT)
    perm = np.concatenate([np.arange(0, D, 2), np.arange(1, D, 2)])  # de-interleave
    wq, wk, wv = w_qkv[0:C], w_qkv[C: 2 * C], w_qkv[2 * C: 3 * C]
    scale = np.float32(D ** -0.5)
    wpT = np.ascontiguousarray(w_proj.T)  # [in_features, out_channels]

    in_maps = []
    for c in range(NCORES):
        b, g = divmod(c, 4)
        h0, h1, h2 = 3 * g, 3 * g + 1, 3 * g + 2

        def qrow(h):
            return wq[h * D: (h + 1) * D][perm] * scale

        def krow(h):
            return wk[h * D: (h + 1) * D][perm]

        def vrow(h):
            return wv[h * D: (h + 1) * D]

        # packed x: [128, strip, kt, 512] so per-(strip, kt-pair) DMA
        # slices have 2KB contiguous runs per partition row
        xT = x[b].T  # [768, 2048]
        xPk = xT.reshape(KT, 128, NSTRIP, IS).transpose(1, 2, 0, 3)
        xPk = xPk.reshape(128, NSTRIP * KT * IS)

        # packed w: block-major [128, (block, kt, cols)]
        def wblock(rows):  # rows [cols_out, 768] -> [128, KT, cols_out]
            wt = rows.T  # [768, cols]
            return wt.reshape(KT, 128, -1).transpose(1, 0, 2)

        wk01 = wblock(np.concatenate([krow(h0), krow(h1)], axis=0))
        wqk2 = wblock(np.concatenate([qrow(h2), krow(h2)], axis=0))
        wq01 = wblock(np.concatenate([qrow(h0), qrow(h1)], axis=0))
        wv012 = wblock(np.concatenate([vrow(h0), vrow(h1), vrow(h2)], axis=0))
        wPk = np.concatenate(
            [wk01.reshape(128, -1), wqk2.reshape(128, -1),
             wq01.reshape(128, -1), wv012.reshape(128, -1)], axis=1
        )  # [128, 3456]

        wp_rows = np.concatenate(
            [wpT[h0 * D: (h0 + 1) * D], wpT[h1 * D: (h1 + 1) * D],
             0.5 * wpT[h2 * D: (h2 + 1) * D], 0.5 * wpT[h2 * D: (h2 + 1) * D]],
            axis=0,
        )  # [256, C]
        in_maps.append(
            {
                "xP": _bf(xPk),
                "wP": _bf(wPk),
                "wp": _bf(wp_rows),
                "cosT": cosT,
                "sinT": sinT,
            }
        )
    return in_maps


def unshard(core_outs, b_proj):
    """Sum the 4 partial projections per batch, transpose, add bias."""
    b_proj = np.asarray(b_proj, dtype=np.float32)
    out = np.empty((B, N, C), dtype=np.float32)
    for b in range(B):
        acc = np.asarray(core_outs[4 * b], dtype=np.float32).copy()
        for g in range(1, 4):
            acc += np.asarray(core_outs[4 * b + g], dtype=np.float32)
        out[b] = acc.T + b_proj
    return out


_NC_CACHE = {}


def get_nc():
    key = (DEBUG_DUMP,)
    if key not in _NC_CACHE:
        _NC_CACHE[key] = build_nc()
    return _NC_CACHE[key]


def run(inputs, trace=False, **spmd_kwargs):
    """Run on hardware; returns (full_output, BassKernelResults)."""
    nc = get_nc()
    in_maps = make_core_inputs(inputs["x"], inputs["w_qkv"], inputs["w_proj"])
    res = bass_utils.run_bass_kernel_spmd(
        nc, in_maps, core_ids=list(range(NCORES)), trace=trace, **spmd_kwargs
    )
    core_outs = [r["outT"] for r in res.results]
    return unshard(core_outs, inputs["b_proj"]), res


def kernel(x, w_qkv, w_proj, b_proj):
    out, _ = run({"x": x, "w_qkv": w_qkv, "w_proj": w_proj, "b_proj": b_proj})
    return out


# revision 21
# speedup vs baseline: 1.0335x; 1.0335x over previous
"""Trainium2 Bass kernel for nn_Attention_7602092114471.

Full multi-head attention block:
  qkv = x @ w_qkv.T ; split q,k,v into 12 heads of d=64
  q = rope(q * d**-0.5) ; k = rope(k)   (lucidrains interleaved RoPE)
  attn = softmax(q @ k.T) ; out = (attn @ v) reassembled, @ w_proj.T + b_proj

Shapes: x [2, 2048, 768], w_qkv [2304, 768], w_proj [768, 768], b_proj [768].

Sharding: 24 (batch, head) pairs -> 8 cores x 3 heads. Core c handles batch
c//4, heads {3g, 3g+1, 3g+2} with g = c%4. Each core computes its heads'
q/k/v projections, attention, and a partial output projection over its
3 heads' feature columns. The host sums the 4 partial projections per batch
(the tensor-parallel all-reduce, done on host during unshard) and adds bias.

v4 design (v1 218us -> v2 209 -> v3 202 -> this). All matmuls bf16.
  * DMA: inputs are pre-packed on the host so every transfer has >=1.5KB
    contiguous runs per partition row (v3's 256-512B runs collapsed
    per-ring bandwidth ~4x and starved both the prologue and the strip-0
    filler chains). x is packed [128, strip, kt, 512]; w is packed
    block-major [128, block, kt, cols]; transfers are split across rings
    (partition quarters / kt pairs) so the prologue set lands in ~4us.
  * One ACT table set for the whole kernel: the exp-only and ln-only
    table entries are masked during compilation so every Exp AND Ln ACT
    resolves to natural_log_exp_and_others -> exactly one ACT_TABLE_LOAD
    (v2 thrashed 13 loads); softmax 1/L runs on DVE reciprocal in-loop
    and on ScalarE Ln/Exp(-x) in the tail where ScalarE is idle.
  * No swap-projection chains: rope's rotate-half partner via 4 cross-
    partition [32,512] bf16 SBUF copies (DVE 4x rate) off a bf16 copy of
    the projection PSUM; sin sign pattern baked into the host table.
  * v produced token-major directly (x-block stationary, v weight columns
    moving) -> [tok, 3*64] PSUM -> one scatter per block. No PE
    transposes; ones columns prefilled by one gpsimd memset.
  * Scores: one [128, 3, 512] PSUM group (3 banks, double-buffered) per
    key block = 3 MMs + ONE 1536-col exp ACT for all 3 heads.
  * Strip 3's PV runs inside the score stream: h0 steals a retiring
    score-group PSUM buffer after jb14, h1/h2 rotate through the work
    banks as late fillers; only the three jb15 matmuls, the tail norms
    and the last projection remain after the final exp.
"""

import os

import numpy as np

import concourse.bass as bass
import concourse.mybir as mybir
import concourse.tile as tile
from concourse import bacc, bass_utils

# Problem constants (hardcoded per contract; kernel.py must be self-contained).
B = 2
N = 2048
C = 768
H = 12
D = 64
ROPE_THETA = 10000.0
NCORES = 8

F32 = mybir.dt.float32
BF16 = mybir.dt.bfloat16

IS = 512                  # strip width (projections and attention i-strips)
NSTRIP = N // IS          # 4
NJB = N // 128            # 16 key blocks
NJP = NJB // 2            # 8 key-block pairs (v_sb/e pairing layout)
KT = C // 128             # 6 contraction tiles for the projections
EXP_BIAS = -2.0           # constant shift inside exp; cancels in normalization

# packed-w block offsets (in the [128, KT*cols] host layout)
WK_OFF = 0                # k0|k1   KT*128
WQK2_OFF = 768            # q2|k2   KT*128
WQ_OFF = 1536             # q0|q1   KT*128
WV_OFF = 2304             # v0|v1|v2  KT*192
WP_COLS = 3456

DEBUG_DUMP = os.environ.get("K_DEBUG_DUMP", "0") == "1"

# ACT table sets whose presence would split Exp and Ln across different
# tables (one reload per switch); masking them makes both resolve to
# natural_log_exp_and_others.
_MASK_ACT_SETS = ("exp_and_others", "natural_log", "exp_and_friends")


def build_nc():
    """Build the per-core Bass module (same NEFF runs SPMD on all 8 cores)."""
    import concourse.bacc as bacc_mod

    orig_tables = bacc_mod.get_activation_tables

    def patched_tables(arch):
        t = orig_tables(arch)
        return {
            name: (set() if name in _MASK_ACT_SETS else funcs)
            for name, funcs in t.items()
        }

    nc = bacc.Bacc(
        "TRN2",
        target_bir_lowering=False,
        debug=False,
        enable_asserts=False,
    )

    xP = nc.dram_tensor("xP", [128, NSTRIP * KT * IS], BF16, kind="ExternalInput").ap()
    wP = nc.dram_tensor("wP", [128, WP_COLS], BF16, kind="ExternalInput").ap()
    wp = nc.dram_tensor("wp", [256, C], BF16, kind="ExternalInput").ap()
    cosT = nc.dram_tensor("cosT", [128, N], BF16, kind="ExternalInput").ap()
    sinT = nc.dram_tensor("sinT", [128, N], BF16, kind="ExternalInput").ap()
    outT = nc.dram_tensor("outT", [C, N], BF16, kind="ExternalOutput").ap()
    dbg = None
    if DEBUG_DUMP:
        dbg = {
            nm: nc.dram_tensor(f"dbg_{nm}", shp, dt, kind="ExternalOutput").ap()
            for nm, shp, dt in [
                ("q01", [128, N], BF16), ("k01", [128, N], BF16),
                ("qk2d", [128, N], BF16),
                ("v_sb", [128, NJP * 2 * 384], BF16),
                ("e0", [128, NJP * 2 * 3 * IS], BF16),
                ("e1", [128, NJP * 2 * 3 * IS], BF16),
                ("P0", [128, N], BF16), ("P1", [128, N], BF16),
            ]
        }

    bacc_mod.get_activation_tables = patched_tables
    try:
        with tile.TileContext(nc) as tc:
            _kernel_body(tc, nc, xP, wP, wp, cosT, sinT, outT, dbg)
        nc.compile()
    finally:
        bacc_mod.get_activation_tables = orig_tables
    return nc


def _kernel_body(tc, nc, xP, wP, wp, cosT, sinT, outT, dbg=None):
    import contextlib

    Exp = mybir.ActivationFunctionType.Exp
    Ln = mybir.ActivationFunctionType.Ln

    ctx = contextlib.ExitStack()
    with ctx:
        persist = ctx.enter_context(tc.tile_pool(name="persist", bufs=1))
        rope_pool = ctx.enter_context(tc.tile_pool(name="rope", bufs=2))
        nrm = ctx.enter_context(tc.tile_pool(name="nrm", bufs=1))
        prout = ctx.enter_context(tc.tile_pool(name="prout", bufs=6))
        attnA = ctx.enter_context(tc.tile_pool(name="attnA", bufs=1))
        # PSUM: 3-bank score groups double-buffered (6) + 2 work banks
        stsp = ctx.enter_context(tc.tile_pool(name="sts", bufs=2, space="PSUM"))
        wkps = ctx.enter_context(tc.tile_pool(name="wkps", bufs=2, space="PSUM"))

        # ---- persistent SBUF tensors -------------------------------------
        q01 = persist.tile([128, N], BF16, name="q01")
        k01 = persist.tile([128, N], BF16, name="k01")
        qk2d = persist.tile([128, N], BF16, name="qk2d")  # q2 rows 0:64 | k2 64:128
        k2lo = persist.tile([64, N], BF16, name="k2lo")   # k2 at base partition 0
        # (v | ones) stationary groups, key-block-paired layout
        v_sb = persist.tile([128, NJP, 2, 3, 128], BF16, name="v_sb")
        P0 = persist.tile([128, N], BF16, name="P0")  # heads h0 | h1
        P1 = persist.tile([128, N], BF16, name="P1")  # h2 duplicated
        wp_sb = persist.tile([128, 2, C], BF16, name="wp_sb")
        bias_sb = persist.tile([128, 1], F32, name="bias_sb")
        warm = persist.tile([128, 64], F32, name="warm")
        warm_o = persist.tile([128, 64], F32, name="warm_o")

        e_all = [
            attnA.tile([128, NJP, 2, 3, IS], BF16, name="e0"),
            attnA.tile([128, NJP, 2, 3, IS], BF16, name="e1"),
        ]

        ph1_stack = contextlib.ExitStack()
        ph1 = ph1_stack.enter_context(tc.tile_pool(name="ph1", bufs=1))
        wk_sb = ph1.tile([128, KT, 128], BF16, name="wk_sb")
        wqk2_sb = ph1.tile([128, KT, 128], BF16, name="wqk2_sb")
        wq_sb = ph1.tile([128, KT, 128], BF16, name="wq_sb")
        wv_sb = ph1.tile([128, KT, 192], BF16, name="wv_sb")
        cos_sb = ph1.tile([128, N], BF16, name="cos_sb")
        sin_sb = ph1.tile([128, N], BF16, name="sin_sb")
        x_sb = ph1.tile([128, NSTRIP, KT, IS], BF16, name="x_sb")

        xPr = xP.rearrange("p (s k c) -> p s k c", s=NSTRIP, k=KT)

        def dma_wblock(dst, off, wd, kt_split=False):
            src = wP[:, off:off + KT * wd].rearrange("p (k c) -> p k c", k=KT)
            if kt_split:
                for j in range(KT // 2):
                    js = slice(2 * j, 2 * j + 2)
                    nc.sync.dma_start(dst[:, js], src[:, js])
                return
            for q in range(4):
                qs = slice(q * 32, (q + 1) * 32)
                nc.sync.dma_start(dst[qs], src[qs])

        def dma_xstrip(s, split_part=False):
            for j in range(KT // 2):
                js = slice(2 * j, 2 * j + 2)
                if split_part:
                    for hx in range(4):
                        hs = slice(hx * 32, (hx + 1) * 32)
                        nc.sync.dma_start(x_sb[hs, s, js], xPr[hs, s, js])
                else:
                    nc.sync.dma_start(x_sb[:, s, js], xPr[:, s, js])

        def dma_trig(s):
            ss = slice(s * IS, (s + 1) * IS)
            for hx in range(2):
                hs = slice(hx * 64, (hx + 1) * 64)
                nc.sync.dma_start(cos_sb[hs, ss], cosT[hs, ss])
                nc.sync.dma_start(sin_sb[hs, ss], sinT[hs, ss])

        # prologue-critical first, in chain consumption order: the k01
        # chain's kt-pair weights + x quarters land on the earliest rings so
        # its matmuls pace with DMA arrival instead of waiting for the set
        wkr = wP[:, WK_OFF:WK_OFF + KT * 128].rearrange("p (k c) -> p k c", k=KT)
        for j in range(KT // 2):
            js = slice(2 * j, 2 * j + 2)
            nc.sync.dma_start(wk_sb[:, js], wkr[:, js])
            for hx in range(4):
                hs = slice(hx * 32, (hx + 1) * 32)
                nc.sync.dma_start(x_sb[hs, 0, js], xPr[hs, 0, js])
        dma_wblock(wqk2_sb, WQK2_OFF, 128, kt_split=True)
        dma_wblock(wq_sb, WQ_OFF, 128, kt_split=True)
        dma_trig(0)
        for s in range(1, NSTRIP):
            dma_xstrip(s)
            dma_trig(s)
        dma_wblock(wv_sb, WV_OFF, 192)
        wpr = wp.rearrange("(o p) f -> p o f", p=128)
        for hx in range(2):
            hs = slice(hx * 64, (hx + 1) * 64)
            nc.sync.dma_start(wp_sb[hs], wpr[hs])

        nc.vector.memset(bias_sb, EXP_BIAS)
        nc.vector.memset(warm, 0.0)
        # early ACT table load during the DMA window
        nc.scalar.activation(out=warm_o, in_=warm, func=Exp)

        # ones columns of the (v | ones) PV groups
        nc.gpsimd.memset(v_sb[:, :, :, :, 64:128], 1.0)

        # ---- projection chain + rope ------------------------------------
        def rope_group(dst, wsrc, s, pre_scalar=False, pt=None, mul_eng=None):
            """One 128-feature projection chain + rope into dst[:, strip s].

            rotate-half partner via 4 cross-partition bf16 copies; sin sign
            pattern ([-sin;+sin] per 32-row half) baked into sinT."""
            ss = slice(s * IS, (s + 1) * IS)
            if pt is None:
                pt = wkps.tile([128, IS], F32, name="wk", tag="wk")
                for kt in range(KT):
                    nc.tensor.matmul(
                        pt, wsrc[:, kt, :], x_sb[:, s, kt, :],
                        start=(kt == 0), stop=(kt == KT - 1),
                    )
            qpre = rope_pool.tile([128, IS], BF16, name="qpre", tag="qpre")
            if pre_scalar:
                nc.scalar.copy(out=qpre, in_=pt)
            else:
                nc.vector.tensor_copy(out=qpre, in_=pt)
            qps = rope_pool.tile([128, IS], BF16, name="qps", tag="qps")
            for (a, b) in ((0, 32), (32, 0), (64, 96), (96, 64)):
                nc.vector.tensor_copy(out=qps[a:a + 32, :], in_=qpre[b:b + 32, :])
            tmp1 = rope_pool.tile([128, IS], BF16, name="tmp1", tag="tmp1")
            tmp2 = rope_pool.tile([128, IS], BF16, name="tmp2", tag="tmp2")
            eng = mul_eng or nc.vector
            eng.tensor_mul(out=tmp1, in0=qpre, in1=cos_sb[:, ss])
            eng.tensor_mul(out=tmp2, in0=qps, in1=sin_sb[:, ss])
            eng.tensor_add(out=dst[:, ss], in0=tmp1, in1=tmp2)
            if dst is qk2d:
                # matmul needs lhsT/rhs on the same base partition: keep a
                # base-0 copy of k2 for the h2 score matmuls
                nc.vector.tensor_copy(out=k2lo[:, ss], in_=qk2d[64:128, ss])

        def v_block(tb):
            """v for one 128-token block, token-major: x-block stationary,
            v weight columns moving -> [tok, 3*64] -> scatter into the
            key-block-paired v_sb layout."""
            s, sb = divmod(tb, 4)
            pt = wkps.tile([128, IS], F32, name="wk", tag="wk")
            for kt in range(KT):
                nc.tensor.matmul(
                    pt[:, 0:192],
                    x_sb[:, s, kt, sb * 128:(sb + 1) * 128],
                    wv_sb[:, kt, :],
                    start=(kt == 0), stop=(kt == KT - 1),
                )
            nc.vector.tensor_copy(
                out=v_sb[:, tb // 2, tb % 2, :, 0:64],
                in_=pt[:, 0:192].rearrange("p (h x) -> p h x", h=3),
            )

        # ---- scores + exp -----------------------------------------------
        def score_group(s, jb):
            ss = slice(s * IS, (s + 1) * IS)
            jbs = slice(jb * 128, (jb + 1) * 128)
            st = stsp.tile([128, 3, IS], F32, name="st", tag="st")
            nc.tensor.matmul(st[:, 0, :], k01[0:64, jbs], q01[0:64, ss],
                             start=True, stop=True)
            nc.tensor.matmul(st[:, 1, :], k01[64:128, jbs], q01[64:128, ss],
                             start=True, stop=True)
            nc.tensor.matmul(st[:, 2, :], k2lo[:, jbs], qk2d[0:64, ss],
                             start=True, stop=True)
            if s == 3 and jb == NJB - 1:
                for h in range(3):
                    nc.scalar.activation(
                        out=e_all[s % 2][:, jb // 2, jb % 2, h], in_=st[:, h, :],
                        func=Exp, bias=bias_sb[:, :],
                    )
            else:
                nc.scalar.activation(
                    out=e_all[s % 2][:, jb // 2, jb % 2], in_=st,
                    func=Exp, bias=bias_sb[:, :],
                )

        # ---- PV + normalization -----------------------------------------
        pvst = {}
        ALLGM = [(g, m) for g in range(NJP) for m in range(2)]

        def pv_mms(ps, h, pv, gms):
            et = e_all[ps % 2]
            for (g, m) in gms:
                nc.tensor.matmul(
                    pv, v_sb[:, g, m, h, :], et[:, g, m, h, :],
                    start=(g == 0 and m == 0),
                    stop=(g == NJP - 1 and m == 1),
                )

        def pv_start(ps, h):
            pv = wkps.tile([128, IS], F32, name="wk", tag="wk")
            pvst[(ps, h)] = pv
            pv_mms(ps, h, pv, ALLGM[:8])

        def pv_end(ps, h):
            pv_mms(ps, h, pvst[(ps, h)], ALLGM[8:])

        def norm01(ps, tail=False):
            """h0/h1: numerators+denominators copied out packed (fast PSUM
            bank release), 1/L via ScalarE Ln -> Exp(-x) (the Ln/Exp pair
            lives in the same ACT table set as the score exps)."""
            ss = slice(ps * IS, (ps + 1) * IS)
            pv0 = pvst.pop((ps, 0))
            pv1 = pvst.pop((ps, 1))
            r01 = nrm.tile([128, IS], F32, name="r01", tag="r01")
            lt = nrm.tile([128, IS], F32, name="lt", tag="lt")
            if tail:
                # no bank pressure after the last strip: read PSUM directly
                cn, cd = None, None
                nc.scalar.activation(out=lt[0:64, :], in_=pv0[64:128, :], func=Ln)
                nc.scalar.activation(out=lt[64:128, :], in_=pv1[64:128, :], func=Ln)
            else:
                cn = nrm.tile([128, IS], F32, name="cn", tag="cn")
                cd = nrm.tile([128, IS], F32, name="cd", tag="cd")
                nc.vector.tensor_copy(out=cn[0:64, :], in_=pv0[0:64, :])
                nc.vector.tensor_copy(out=cd[0:64, :], in_=pv0[64:128, :])
                nc.vector.tensor_copy(out=cn[64:128, :], in_=pv1[0:64, :])
                nc.vector.tensor_copy(out=cd[64:128, :], in_=pv1[64:128, :])
                nc.scalar.activation(out=lt, in_=cd, func=Ln)
            nc.scalar.activation(out=r01, in_=lt, func=Exp, scale=-1.0)
            n0 = pv0[0:64, :] if tail else cn[0:64, :]
            n1 = pv1[0:64, :] if tail else cn[64:128, :]
            nc.vector.tensor_mul(out=P0[0:64, ss], in0=n0, in1=r01[0:64, :])
            nc.vector.tensor_mul(out=P0[64:128, ss], in0=n1, in1=r01[64:128, :])

        def norm2(ps, tail=False):
            ss = slice(ps * IS, (ps + 1) * IS)
            pv2 = pvst.pop((ps, 2))
            r2 = nrm.tile([64, IS], F32, name="r2", tag="r2")
            t2 = nrm.tile([64, IS], F32, name="t2", tag="t2")
            if tail:
                cn2 = None
                nc.scalar.activation(out=t2, in_=pv2[64:128, :], func=Ln)
            else:
                cn2 = nrm.tile([64, IS], F32, name="cn2", tag="cn2")
                cd2 = nrm.tile([64, IS], F32, name="cd2", tag="cd2")
                nc.vector.tensor_copy(out=cn2, in_=pv2[0:64, :])
                nc.vector.tensor_copy(out=cd2, in_=pv2[64:128, :])
                nc.scalar.activation(out=t2, in_=cd2, func=Ln)
            nc.scalar.activation(out=r2, in_=t2, func=Exp, scale=-1.0)
            n2 = pv2[0:64, :] if tail else cn2
            nc.vector.tensor_mul(out=P1[0:64, ss], in0=n2, in1=r2)
            nc.vector.tensor_copy(out=P1[64:128, ss], in_=P1[0:64, ss])

        def proj_obs(t, obs, alt_cast=False):
            ss = slice(t * IS, (t + 1) * IS)
            for ob in obs:
                obsl = slice(ob * 128, (ob + 1) * 128)
                pp = wkps.tile([128, IS], F32, name="wk", tag="wk")
                nc.tensor.matmul(pp, wp_sb[:, 0, obsl], P0[:, ss],
                                 start=True, stop=False)
                nc.tensor.matmul(pp, wp_sb[:, 1, obsl], P1[:, ss],
                                 start=False, stop=True)
                ot = prout.tile([128, IS], BF16, name="ot", tag="ot")
                if alt_cast and ob % 2 == 1:
                    nc.scalar.copy(out=ot, in_=pp)
                else:
                    nc.vector.tensor_copy(out=ot, in_=pp)
                for hx in range(2):
                    hs = slice(ob * 128 + hx * 64, ob * 128 + (hx + 1) * 64)
                    nc.sync.dma_start(outT[hs, ss], ot[hx * 64:(hx + 1) * 64, :])

        # ---- prologue: strip 0's own projections ------------------------
        rope_group(k01, wk_sb, 0, pre_scalar=True)
        rope_group(qk2d, wqk2_sb, 0, pre_scalar=True)
        rope_group(q01, wq_sb, 0, pre_scalar=True)

        # ---- strip 0: scores/exp with phase 1 as filler ------------------
        # k01/qk2 of strip t must land before score group jb=4t.
        s0_fillers = [
            lambda: rope_group(k01, wk_sb, 1),
            lambda: rope_group(qk2d, wqk2_sb, 1),
            lambda: rope_group(k01, wk_sb, 2),
            lambda: rope_group(qk2d, wqk2_sb, 2),
            lambda: rope_group(k01, wk_sb, 3),
            lambda: rope_group(qk2d, wqk2_sb, 3),
            lambda: v_block(0),
            lambda: v_block(1),
            lambda: rope_group(q01, wq_sb, 1),
            lambda: v_block(2),
            lambda: v_block(3),
            lambda: rope_group(q01, wq_sb, 2),
            lambda: v_block(4),
            lambda: v_block(5),
            lambda: v_block(6),
            lambda: v_block(7),
        ]
        for jb in range(NJB):
            score_group(0, jb)
            s0_fillers[jb]()

        # ---- strips 1..3 + PV/norm/proj fillers + tail -------------------
        def pv3_main(h):
            pv = wkps.tile([128, IS], F32, name="wk", tag="wk")
            pvst[(3, h)] = pv
            pv_mms(3, h, pv, ALLGM[:15])

        strip_fillers = {
            1: [
                lambda: rope_group(q01, wq_sb, 3),
                lambda: v_block(8), lambda: v_block(9),
                lambda: v_block(10), lambda: v_block(11),
                lambda: pv_start(0, 0),
                lambda: v_block(12), lambda: v_block(13),
                lambda: v_block(14), lambda: v_block(15),
                lambda: pv_end(0, 0),
                lambda: pv_start(0, 1), lambda: pv_end(0, 1),
                lambda: norm01(0),
                lambda: (pv_start(0, 2), pv_end(0, 2)),
                lambda: norm2(0),
            ],
            2: [
                lambda: pv_start(1, 0), lambda: pv_end(1, 0),
                lambda: pv_start(1, 1), lambda: pv_end(1, 1),
                lambda: norm01(1),
                lambda: pv_start(1, 2), lambda: pv_end(1, 2),
                lambda: norm2(1),
                lambda: proj_obs(0, [0, 1]),
                lambda: proj_obs(0, [2, 3]),
                lambda: proj_obs(0, [4, 5]),
            ],
            3: [
                lambda: pv_start(2, 0), lambda: pv_end(2, 0),
                lambda: pv_start(2, 1), lambda: pv_end(2, 1),
                lambda: norm01(2),
                lambda: pv_start(2, 2), lambda: pv_end(2, 2),
                lambda: norm2(2),
                lambda: proj_obs(1, [0, 1]),
                lambda: proj_obs(1, [2, 3]),
                lambda: proj_obs(1, [4, 5]),
                lambda: proj_obs(2, [0, 1]),
                lambda: proj_obs(2, [2, 3]),
                lambda: proj_obs(2, [4, 5]),
                lambda: pv3_main(1),
                lambda: pv3_main(2),
            ],
        }
        for s in range(1, NSTRIP):
            fillers = strip_fillers[s]
            fi = 0
            for jb in range(NJB):
                score_group(s, jb)
                if s == 3 and jb == 14:
                    # steal the score-group PSUM buffer retiring after
                    # ACT(13) so h0's PV overlaps the last score groups
                    st3 = stsp.tile([128, 3, IS], F32, name="st", tag="st")
                    pvst[(3, 0)] = st3[:, 0, :]
                    pv_mms(3, 0, pvst[(3, 0)], ALLGM[:15])
                if fi < len(fillers):
                    fillers[fi]()
                    fi += 1
            while fi < len(fillers):
                fillers[fi]()
                fi += 1
            if s == 1:
                ph1_stack.close()

        # tail: only the jb15 PV matmuls, tail norms, last projection
        pv_mms(3, 0, pvst[(3, 0)], [ALLGM[15]])
        pv_mms(3, 1, pvst[(3, 1)], [ALLGM[15]])
        pv_mms(3, 2, pvst[(3, 2)], [ALLGM[15]])
        norm01(3, tail=True)
        norm2(3, tail=True)
        # the score-group PSUM buffers are retired: 2 steals = 6 banks, so
        # the six tail projections never wait on a bank rotation
        stp = [stsp.tile([128, 3, IS], F32, name="st", tag="st") for _ in range(2)]
        ss3 = slice(3 * IS, 4 * IS)
        for ob in range(6):
            obsl = slice(ob * 128, (ob + 1) * 128)
            pp = stp[ob // 3][:, ob % 3, :]
            nc.tensor.matmul(pp, wp_sb[:, 0, obsl], P0[:, ss3],
                             start=True, stop=False)
            nc.tensor.matmul(pp, wp_sb[:, 1, obsl], P1[:, ss3],
                             start=False, stop=True)
            ot = prout.tile([128, IS], BF16, name="ot", tag="ot")
            if ob % 2 == 1:
                nc.scalar.copy(out=ot, in_=pp)
            else:
                nc.vector.tensor_copy(out=ot, in_=pp)
            for hx in range(2):
                hs = slice(ob * 128 + hx * 64, ob * 128 + (hx + 1) * 64)
                nc.sync.dma_start(outT[hs, ss3], ot[hx * 64:(hx + 1) * 64, :])

        if dbg is not None:
            nc.sync.dma_start(dbg["q01"], q01)
            nc.sync.dma_start(dbg["k01"], k01)
            nc.sync.dma_start(dbg["qk2d"], qk2d)
            nc.sync.dma_start(dbg["v_sb"], v_sb.rearrange("p a b c d -> p (a b c d)"))
            nc.sync.dma_start(dbg["e0"], e_all[0].rearrange("p a b c d -> p (a b c d)"))
            nc.sync.dma_start(dbg["e1"], e_all[1].rearrange("p a b c d -> p (a b c d)"))
            nc.sync.dma_start(dbg["P0"], P0)
            nc.sync.dma_start(dbg["P1"], P1)


# ---------------------------------------------------------------------------
# Host-side sharding / unsharding
# ---------------------------------------------------------------------------

def _rope_tables():
    inv_freq = 1.0 / (ROPE_THETA ** (np.arange(0, D, 2, dtype=np.float64) / D))
    ang = np.arange(N, dtype=np.float64)[None, :] * inv_freq[:, None]  # [32, N]
    cos64 = np.concatenate([np.cos(ang), np.cos(ang)], axis=0)
    sin64 = np.concatenate([-np.sin(ang), np.sin(ang)], axis=0)
    cosT = np.concatenate([cos64, cos64], axis=0)
    sinT = np.concatenate([sin64, sin64], axis=0)
    return cosT, sinT  # [128, N] float64


def _bf(a):
    import ml_dtypes

    return np.ascontiguousarray(a).astype(ml_dtypes.bfloat16)


def make_core_inputs(x, w_qkv, w_proj):
    """Build the 8 per-core input dicts from full inputs."""
    x = np.asarray(x, dtype=np.float32)
    w_qkv = np.asarray(w_qkv, dtype=np.float32)
    w_proj = np.asarray(w_proj, dtype=np.float32)

    cosT, sinT = _rope_tables()
    cosT, sinT = _bf(cosT), _bf(sinT)
    perm = np.concatenate([np.arange(0, D, 2), np.arange(1, D, 2)])  # de-interleave
    wq, wk, wv = w_qkv[0:C], w_qkv[C: 2 * C], w_qkv[2 * C: 3 * C]
    scale = np.float32(D ** -0.5)
    wpT = np.ascontiguousarray(w_proj.T)  # [in_features, out_channels]

    in_maps = []
    for c in range(NCORES):
        b, g = divmod(c, 4)
        h0, h1, h2 = 3 * g, 3 * g + 1, 3 * g + 2

        def qrow(h):
            return wq[h * D: (h + 1) * D][perm] * scale

        def krow(h):
            return wk[h * D: (h + 1) * D][perm]

        def vrow(h):
            return wv[h * D: (h + 1) * D]

        # packed x: [128, strip, kt, 512] so per-(strip, kt-pair) DMA
        # slices have 2KB contiguous runs per partition row
        xT = x[b].T  # [768, 2048]
        xPk = xT.reshape(KT, 128, NSTRIP, IS).transpose(1, 2, 0, 3)
        xPk = xPk.reshape(128, NSTRIP * KT * IS)

        # packed w: block-major [128, (block, kt, cols)]
        def wblock(rows):  # rows [cols_out, 768] -> [128, KT, cols_out]
            wt = rows.T  # [768, cols]
            return wt.reshape(KT, 128, -1).transpose(1, 0, 2)

        wk01 = wblock(np.concatenate([krow(h0), krow(h1)], axis=0))
        wqk2 = wblock(np.concatenate([qrow(h2), krow(h2)], axis=0))
        wq01 = wblock(np.concatenate([qrow(h0), qrow(h1)], axis=0))
        wv012 = wblock(np.concatenate([vrow(h0), vrow(h1), vrow(h2)], axis=0))
        wPk = np.concatenate(
            [wk01.reshape(128, -1), wqk2.reshape(128, -1),
             wq01.reshape(128, -1), wv012.reshape(128, -1)], axis=1
        )  # [128, 3456]

        wp_rows = np.concatenate(
            [wpT[h0 * D: (h0 + 1) * D], wpT[h1 * D: (h1 + 1) * D],
             0.5 * wpT[h2 * D: (h2 + 1) * D], 0.5 * wpT[h2 * D: (h2 + 1) * D]],
            axis=0,
        )  # [256, C]
        in_maps.append(
            {
                "xP": _bf(xPk),
                "wP": _bf(wPk),
                "wp": _bf(wp_rows),
                "cosT": cosT,
                "sinT": sinT,
            }
        )
    return in_maps


def unshard(core_outs, b_proj):
    """Sum the 4 partial projections per batch, transpose, add bias."""
    b_proj = np.asarray(b_proj, dtype=np.float32)
    out = np.empty((B, N, C), dtype=np.float32)
    for b in range(B):
        acc = np.asarray(core_outs[4 * b], dtype=np.float32).copy()
        for g in range(1, 4):
            acc += np.asarray(core_outs[4 * b + g], dtype=np.float32)
        out[b] = acc.T + b_proj
    return out


_NC_CACHE = {}


def get_nc():
    key = (DEBUG_DUMP,)
    if key not in _NC_CACHE:
        _NC_CACHE[key] = build_nc()
    return _NC_CACHE[key]


def run(inputs, trace=False, **spmd_kwargs):
    """Run on hardware; returns (full_output, BassKernelResults)."""
    nc = get_nc()
    in_maps = make_core_inputs(inputs["x"], inputs["w_qkv"], inputs["w_proj"])
    res = bass_utils.run_bass_kernel_spmd(
        nc, in_maps, core_ids=list(range(NCORES)), trace=trace, **spmd_kwargs
    )
    core_outs = [r["outT"] for r in res.results]
    return unshard(core_outs, inputs["b_proj"]), res


def kernel(x, w_qkv, w_proj, b_proj):
    out, _ = run({"x": x, "w_qkv": w_qkv, "w_proj": w_proj, "b_proj": b_proj})
    return out
